# revision 46
# baseline (speedup 1.0000x reference)
"""Trainium2 Bass kernel for nn_DynamicConv (per-pixel dynamic 5x5 conv, 8 heads).

Reference computation (per batch image b):
    f[i, j, :]  = sum_c x[b, c, i, j] * filt_w[c, :]          # (56,56,200)
    out[c, i, j] = sum_{kh,kw} xpad[c, i+kh, j+kw] * f[i, j, kh, kw, c//24]

Sharding: data-parallel over batch, but each core takes 2 images x one
28-column half of the width so that 112 of 128 SBUF partitions carry
(row, image) pairs: partition q = 2*row + img.  Compute-engine APs must
start at partition 0 (quad-aligned), so the five kh row shifts are
materialized as five separately-laid-out DRAM loads x_d0..x_d4
(x_dk[q, c, jp] = xpad[img, c, i+k, jp]); the kw shifts are free-dim
offsets.

VERSION=8 (fp16 pipeline, measured 108us vs the 256us fp32 v4 baseline;
v4 kept for reference):
 - Everything 16-bit on chip: x loads, filt weights, generated filters,
   products, and the output store are float16; only PSUM accumulation
   (exact) is fp32.  DVE tensor ops hit the 2x_1p fast path (2-byte
   packed operands, even innermost counts) and PE matmuls run 1
   cycle/col.  Measured absmax rel err 6.5e-4 (harness gate 2e-2).
 - DMA: transfers are chained in need order (chain_iter_dep) because
   concurrent transfers round-robin at ~1/N bandwidth each; x_gen is
   split unevenly (8/20 j columns) so its small first piece exits the
   round-robin early and filter-gen starts on it.
 - PE filter-gen: per output column j, the (96ch x 128px) channel-major
   x slice is stationary (128 cols => automatic Fast Weight Load)
   against fw columns, fp32 PSUM batched 4 j per tile so evacuation
   writes contiguous fp16 runs (scattered stride-28 writes cost
   ~5ns/elem).  Split by head-half: phase A (heads 0-3) gates the conv
   start, phase B interleaves with the first conv half on the PE's
   slack (region-level dep tracking).
 - DVE conv: one (head, ch, j) product tap per channel-half (walrus
   limits TensorTensor to 3 free dims, and the f-broadcast across the
   24 channels of a head needs its own step-0 dim): 50 tensor_muls of
   2688 elems at ~1514ns, 75.7us total -- the kernel's floor.
 - PE sums the 25 taps per half into PSUM via fp16 identity matmuls
   (6 x 448-col chunks, 112-row identity padded to 128 cols for FWL);
   ACT/DVE split the evacuation; output chunks DMA out as they land.
 - GPSIMD offload rejected twice (see GJ/GT notes below).
"""

import numpy as np

import concourse.bacc as bacc
import concourse.mybir as mybir
import concourse.tile as tile
from concourse.bass_utils import run_bass_kernel_spmd

B, C, H, W = 8, 192, 56, 56
K, HEADS = 5, 8
CG = C // HEADS            # 24 channels per head
FCOLS = K * K * HEADS      # 200 filter-gen outputs per pixel
WH = 28                    # columns per core (half width)
JP = WH + 4                # padded columns held in SBUF
P_O = 2 * H                # 112 partitions carrying (row, img) pairs
JCH = 7                    # filter-gen j-columns per chunk
N_CORES = 8

F32 = mybir.dt.float32
F16 = mybir.dt.float16

VERSION = 8

F32R = mybir.dt.float32r
HHEADS = HEADS // 2        # heads per channel-half
NCH = 6                    # PSUM chunks per half
CHF = 96 * WH // NCH       # 448 fp32 per chunk = 16 channels x 28 cols
# GPSIMD product offload, tried twice and reverted both times:
#  - j-slices: ~1.6us fixed overhead per op (a 480-elem op cost 2.6us), and
#    the DVE's odd-width remainder lost its 2x_1p fast path (1520 -> 2607ns).
#  - whole taps: the op itself matched the model (6.2-6.4us for 2688 elems),
#    but concurrent GPSIMD SBUF reads slowed EVERY DVE multiply by ~28%
#    (1517 -> 1938 ns) via SBUF port contention - a large net loss.
GJ = 0                     # j columns of each product tap done on GPSIMD
GT = 0                     # whole taps per half done on GPSIMD (kh=0 taps)


def build_nc(version=None):
    if version is None:
        version = VERSION
    if version >= 8:
        return build_nc_v8()
    return build_nc_v4(version)


def build_nc_v8():
    nc = bacc.Bacc(None)

    xd_in = [
        nc.dram_tensor(f"x_d{k}", [P_O, C, JP], F16, kind="ExternalInput")
        for k in range(K)
    ]
    # Stationary operands carry 128 columns (output partitions) so the
    # compiler's Fast Weight Load kicks in (requires NumWeights==128 and a
    # non-fp32 dtype); the extra columns are zero and the extra PSUM
    # partitions are never read.  j-major layout so the two chained
    # j-half DMAs are one contiguous run per partition.
    xg_in = nc.dram_tensor("x_gen", [96, WH, 2, 128], F16, kind="ExternalInput")
    fw_in = nc.dram_tensor("fw_pk", [96, 2, FCOLS], F16, kind="ExternalInput")
    id_in = nc.dram_tensor("identh", [P_O, 128], F16, kind="ExternalInput")
    out_d = nc.dram_tensor("out_sbl", [P_O, C, WH], F16, kind="ExternalOutput")

    with tile.TileContext(nc) as tc:
        with (
            tc.tile_pool(name="big", bufs=1) as big,
            tc.tile_pool(name="pr", bufs=4) as pr,
            tc.tile_pool(name="ps_f", bufs=2, space="PSUM") as ps_f,
            tc.tile_pool(name="ps_a", bufs=NCH, space="PSUM") as ps_a,
        ):
            fw_sb = big.tile([96, 2, FCOLS], F16)
            xg = big.tile([96, WH, 2, 128], F16)
            xd = [
                big.tile([P_O, C, JP], F16, tag=f"xd{k}", name=f"xd{k}")
                for k in range(K)
            ]
            f_sb = big.tile([P_O, K * K, HEADS, WH], F16)
            acc = big.tile([P_O, C, WH], F16)
            ident = big.tile([P_O, 128], F16)

            # SDMA engines round-robin between logical queues at packet
            # granularity, so concurrent transfers each get ~1/N bandwidth
            # and equal-size splits all finish together, while chaining
            # costs ~2.7us dead time per hop (completion sem + reissue).
            # So: split x_gen UNEVENLY (8/20 columns), both issued at t=0 -
            # round-robin lets the small first piece exit early (filter-gen
            # phase A starts on it) without delaying the total - and chain
            # the five xd loads in need order behind the small piece.
            XJ0 = 8            # covers the first two 4-column filt-gen groups
                               # (12 and chaining xd0 behind all of xg both
                               # measured ~2-4us worse: this DMA schedule is
                               # a tight local minimum)
            nc.sync.dma_start(fw_sb[:], fw_in[:])
            nc.sync.dma_start(ident[:], id_in[:])
            i_xg0 = nc.sync.dma_start(xg[:, :XJ0], xg_in[:, :XJ0])
            tc.chain_iter_dep("dma_b1", i_xg0.ins)
            tc.chain_iter_dep("dma_b2", i_xg0.ins)
            nc.sync.dma_start(xg[:, XJ0:], xg_in[:, XJ0:])
            # xd0 alone first (first conv tap gates on it; chaining it
            # behind ALL of xg measured worse), then two parallel branches
            # {xd1->xd3} and {xd2->xd4}: fewer 2.7us chain hops while each
            # arrival still beats its first tap.
            i_xd0 = nc.sync.dma_start(xd[0][:], xd_in[0][:])
            tc.chain_iter_dep("dma_b1", i_xd0.ins)
            tc.chain_iter_dep("dma_b2", i_xd0.ins)
            for k, key in [(1, "dma_b1"), (2, "dma_b2"), (3, "dma_b1"), (4, "dma_b2")]:
                i_k = nc.sync.dma_start(xd[k][:], xd_in[k][:])
                tc.chain_iter_dep(key, i_k.ins)

            # PE warm-up: dummy matmuls on the already-resident identity
            # during the x_gen DMA wait pre-ramp the HAM clock gate (full
            # 2.4GHz needs ~3us of continuous PE busy), so filter-gen
            # phase A doesn't run at the 1.2GHz mid p-state.  Sized to end
            # just before x_gen's first piece lands (~12.4us).
            for w in range(22):
                wps = ps_f.tile([128, 128], F32, tag="fps", name=f"warm{w}")
                nc.tensor.matmul(wps[:], ident[:], ident[:], start=True, stop=True)

            # ---- filter generation: f[q, kl, h, j] = sum_c x[c, q, j] * fw[c, kl*8+h]
            # FJB j columns accumulate into one PSUM tile (j-slot major, so
            # each matmul writes a contiguous run) and evacuate together:
            # the f_sb write then lands as contiguous fp16 runs instead of
            # lone stride-28 elements, which dominated v9's head (~1us per
            # single-column copy).
            #
            # Filter generation is split by head-half: phase A (heads 0-3)
            # gates the conv start; phase B (heads 4-7) interleaves with the
            # first conv half on the PE's slack (region-level dependency
            # tracking lets hh=0 products start on phase A alone).
            FJB = 4            # j columns batched per filter-gen PSUM tile
            def filt_gen(hp, jc, evac_eng):
                h0 = hp * HHEADS
                fps = ps_f.tile([128, FJB, K * K * HHEADS], F32, tag="fps")
                fwv = fw_sb[:].rearrange("c k (kl h) -> c k kl h", kl=K * K)
                for js in range(FJB):
                    for ck in range(2):
                        nc.tensor.matmul(
                            fps[:, js, :],
                            xg[:, FJB * jc + js, ck, :],  # (96 ch, 128 px) stationary
                            fwv[:, ck, :, h0 : h0 + HHEADS],  # (96 ch, 25*4)
                            start=(ck == 0),
                            stop=(ck == 1),
                        )
                src = fps[:P_O].rearrange("p s (kl h) -> p kl h s", kl=K * K)
                dst = f_sb[:, :, h0 : h0 + HHEADS, FJB * jc : FJB * (jc + 1)]
                if evac_eng == "act":
                    nc.scalar.copy(dst, src)
                else:
                    nc.vector.tensor_copy(dst, src)

            # the LAST group's evac goes on the DVE: the first conv product
            # (also DVE) then queues right behind it with no cross-engine
            # semaphore hop
            for jc in range(WH // FJB):
                filt_gen(0, jc, "act" if jc % 2 == 1 else "dve")

            # ---- conv: DVE computes one (h, g, j) product tap per half
            # (ISA limit: TensorTensor free APs are at most 3-D, and the
            # g-broadcast of f needs its own step-0 dim); PE accumulates
            # all 25 taps into PSUM.  GT whole taps per half run on the
            # otherwise-idle GPSIMD into dedicated tiles; their PE
            # accumulation is spliced late in each half's stream so a
            # slower-than-expected GPSIMD never stalls the PE.
            def tap_views(hh, kl):
                kh, kw = divmod(kl, K)
                c0 = hh * 96
                xin = xd[kh][:, c0 : c0 + 96, kw : kw + WH]
                xin4 = xin.rearrange("p (h g) j -> p h g j", h=HHEADS)
                fv = (
                    f_sb[:, kl, hh * HHEADS : (hh + 1) * HHEADS, :]
                    .unsqueeze(2)
                    .broadcast_to([P_O, HHEADS, CG, WH])
                )
                return xin4, fv

            gp_taps = list(range(GT))        # kh=0 taps: ready earliest
            gp_prod = {}

            def gp_issue(hh):
                for kl in gp_taps:
                    t = big.tile(
                        [P_O, HHEADS, CG, WH], F16, name=f"gprod{hh}_{kl}"
                    )
                    xin4, fv = tap_views(hh, kl)
                    nc.gpsimd.tensor_mul(t[:], xin4, fv)
                    gp_prod[(hh, kl)] = t

            def accum(accps, prod, first, last):
                pm = prod[:].rearrange("p h g j -> p (h g j)")
                for b in range(NCH):
                    nc.tensor.matmul(
                        accps[b][:],
                        ident[:],
                        pm[:, b * CHF : (b + 1) * CHF],
                        start=first,
                        stop=last,
                    )

            gp_issue(0)
            for hh in range(2):
                c0 = hh * 96
                accps = [
                    ps_a.tile([128, CHF], F32, tag="accps", name=f"accps{hh}_{b}")
                    for b in range(NCH)
                ]
                dve_taps = [kl for kl in range(K * K) if kl not in gp_taps]
                splice_at = 18 if hh == 0 else 16
                for idx, kl in enumerate(dve_taps):
                    xin4, fv = tap_views(hh, kl)
                    prod = pr.tile(
                        [P_O, HHEADS, CG, WH], F16, tag="prod",
                        name=f"prod{hh}_{kl}",
                    )
                    nc.vector.tensor_mul(prod[:], xin4, fv)

                    # interleave phase-B filter generation (heads 4-7) with
                    # the first conv half, every other tap so the added PE
                    # work (8 matmuls/group) stays under the DVE tap pace;
                    # its evacs stay off the busy DVE
                    if hh == 0 and idx % 2 == 0 and idx // 2 < WH // FJB:
                        filt_gen(1, idx // 2, "act")

                    accum(accps, prod, idx == 0, idx == len(dve_taps) - 1)
                    if idx == splice_at:
                        for kl_g in gp_taps:
                            accum(accps, gp_prod[(hh, kl_g)], False, False)
                # phase B is fully emitted by the end of the hh=0 loop, so
                # the hh=1 GPSIMD products (which read phase-B f regions)
                # can only be issued here.
                if hh == 0:
                    gp_issue(1)
                # PSUM evacuation per 16-channel chunk, store per 48-channel
                # group: 16-channel chunk DMAs were 112 descriptors of 896B
                # each and the final six drained for ~4us; a 48-channel
                # group is one contiguous 2688B run per partition (6x fewer
                # descriptors).  The final half interleaves ACT/DVE per
                # group so each group's three evacs (and so its DMA) finish
                # as early as possible.
                # final half: DVE (free after its last product, and faster
                # per copy) takes b0/b2 and crucially b5 - the copy gating
                # the last output DMA
                dve_evacs = {0, 2, 5}
                for g in range(2):
                    for i in range(NCH // 2):
                        b = g * (NCH // 2) + i
                        dst = acc[:, c0 + b * 16 : c0 + (b + 1) * 16, :]
                        src = accps[b][:P_O].rearrange("p (c j) -> p c j", j=WH)
                        if hh == 1 and b in dve_evacs:
                            nc.vector.tensor_copy(dst, src)
                        else:
                            nc.scalar.copy(dst, src)
                    lo = c0 + g * 48
                    nc.sync.dma_start(
                        out_d[:, lo : lo + 48, :], acc[:, lo : lo + 48, :]
                    )

    return nc


def build_nc_v4(version=4):
    nc = bacc.Bacc(None)

    xd_in = [
        nc.dram_tensor(f"x_d{k}", [P_O, C, JP], F32, kind="ExternalInput")
        for k in range(K)
    ]
    xg_in = nc.dram_tensor("x_gen", [96, 2, WH, P_O], F32, kind="ExternalInput")
    fw_in = nc.dram_tensor("fw_pk", [96, 2, FCOLS], F32, kind="ExternalInput")
    id_in = nc.dram_tensor("ident", [P_O, P_O], F32R, kind="ExternalInput")
    out_d = nc.dram_tensor("out_sbl", [P_O, C, WH], F32, kind="ExternalOutput")

    with tile.TileContext(nc) as tc:
        with (
            tc.tile_pool(name="big", bufs=1) as big,
            tc.tile_pool(name="sh", bufs=2) as sh,
            tc.tile_pool(name="ps_f", bufs=2, space="PSUM") as ps_f,
            tc.tile_pool(name="ps_a", bufs=NCH, space="PSUM") as ps_a,
        ):
            xd = [
                big.tile([P_O, C, JP], F32, tag=f"xd{k}", name=f"xd{k}")
                for k in range(K)
            ]
            fw_sb = big.tile([96, 2, FCOLS], F32)
            f_sb = big.tile([P_O, K * K, HEADS, WH], F32)
            acc = big.tile([P_O, C, WH], F32)
            ident = big.tile([P_O, P_O], F32R)

            nc.sync.dma_start(ident[:], id_in[:])
            for k in range(K):
                nc.sync.dma_start(xd[k][:], xd_in[k][:])
            nc.sync.dma_start(fw_sb[:], fw_in[:])

            for jc in range(WH // JCH):
                xg = sh.tile([96, 2, JCH, P_O], F32, tag="xgprod")
                nc.sync.dma_start(xg[:], xg_in[:, :, jc * JCH : (jc + 1) * JCH, :])
                for jl in range(JCH):
                    j = jc * JCH + jl
                    fps = ps_f.tile([P_O, K * K, HEADS], F32, tag="fps")
                    for ck in range(2):
                        nc.tensor.matmul(
                            fps[:],
                            xg[:, ck, jl, :],
                            fw_sb[:, ck, :],
                            start=(ck == 0),
                            stop=(ck == 1),
                        )
                    nc.scalar.copy(f_sb[:, :, :, j], fps[:])

            for hh in range(2):
                c0 = hh * 96
                accps = [
                    ps_a.tile([P_O, CHF], F32, tag="accps", name=f"accps{hh}_{b}")
                    for b in range(NCH)
                ]
                for kl in range(K * K):
                    kh, kw = divmod(kl, K)
                    xin = xd[kh][:, c0 : c0 + 96, kw : kw + WH]
                    xin4 = xin.rearrange("p (h g) j -> p h g j", h=HHEADS)
                    fbc = (
                        f_sb[:, kl, hh * HHEADS : (hh + 1) * HHEADS, :]
                        .unsqueeze(2)
                        .broadcast_to([P_O, HHEADS, CG, WH])
                    )
                    prod = sh.tile(
                        [P_O, 96, WH], F32R, tag="xgprod", name=f"prod{hh}_{kl}",
                    )
                    p4 = prod[:].rearrange("p (h g) j -> p h g j", h=HHEADS)
                    nc.vector.tensor_mul(p4, xin4, fbc)
                    pflat = prod[:].rearrange("p c j -> p (c j)")
                    for b in range(NCH):
                        nc.tensor.matmul(
                            accps[b][:],
                            ident[:],
                            pflat[:, b * CHF : (b + 1) * CHF],
                            start=(kl == 0),
                            stop=(kl == K * K - 1),
                        )
                for b in range(NCH):
                    nc.scalar.copy(
                        acc[:, c0 + b * 16 : c0 + (b + 1) * 16, :],
                        accps[b][:].rearrange("p (c j) -> p c j", j=WH),
                    )

            nc.sync.dma_start(out_d[:], acc[:])

    return nc


def shard_inputs(x, filt_w, version=None):
    """Split full inputs into the 8 per-core input maps."""
    if version is None:
        version = VERSION
    dt = np.float16 if version >= 8 else np.float32
    x = np.ascontiguousarray(np.asarray(x, dtype=np.float32))
    fw = np.ascontiguousarray(np.asarray(filt_w, dtype=np.float32))
    fw_pk = np.ascontiguousarray(
        fw.reshape(2, 96, FCOLS).transpose(1, 0, 2)
    ).astype(dt)

    in_maps = []
    for core in range(N_CORES):
        pair, jh = divmod(core, 2)
        xs = x[2 * pair : 2 * pair + 2]           # (2, C, 56, 56)
        xpad = np.zeros((2, C, H + 4, JP), np.float32)
        lo = jh * WH - 2                           # global col of jp=0
        s_lo, s_hi = max(lo, 0), min(lo + JP, W)
        xpad[:, :, 2 : 2 + H, s_lo - lo : s_lo - lo + (s_hi - s_lo)] = xs[
            :, :, :, s_lo:s_hi
        ]
        m = {"fw_pk": fw_pk}
        if version >= 8:
            m["identh"] = np.eye(P_O, 128, dtype=np.float16)
        else:
            m["ident"] = np.eye(P_O, dtype=np.float32)
        for k in range(K):
            # x_dk[2*i+img, c, jp] = xpad[img, c, i+k, jp]
            m[f"x_d{k}"] = np.ascontiguousarray(
                xpad[:, :, k : k + H, :].transpose(2, 0, 1, 3).reshape(P_O, C, JP)
            ).astype(dt)
        # channel-major copy for filter-gen: x_gen[c96, j, ck, 2*i+img]
        xs_half = xs[:, :, :, jh * WH : (jh + 1) * WH]  # (2, C, 56, 28)
        xg = xs_half.transpose(1, 3, 2, 0).reshape(C, WH, P_O)
        if version >= 8:
            # j-major for the split DMA; pad px to 128 for FWL
            xg = xg.reshape(2, 96, WH, P_O).transpose(1, 2, 0, 3)
            xgp = np.zeros((96, WH, 2, 128), np.float32)
            xgp[:, :, :, :P_O] = xg
            xg = xgp
        else:
            xg = xg.reshape(2, 96, WH, P_O).transpose(1, 0, 2, 3)
        m["x_gen"] = np.ascontiguousarray(xg).astype(dt)
        in_maps.append(m)
    return in_maps


def unshard_output(results):
    """Reassemble the 8 per-core outputs into the full (B, C, H, W) tensor."""
    out = np.empty((B, C, H, W), np.float32)
    for core in range(N_CORES):
        pair, jh = divmod(core, 2)
        arr = np.asarray(results[core]["out_sbl"]).astype(np.float32)
        arr = arr.reshape(H, 2, C, WH)
        # arr[i, img, c, j] = out[2*pair+img, c, i, jh*28+j]
        out[2 * pair : 2 * pair + 2, :, :, jh * WH : (jh + 1) * WH] = arr.transpose(
            1, 2, 0, 3
        )
    return out


_NC_CACHE = {}


def _get_nc(version=None):
    if version is None:
        version = VERSION
    if version not in _NC_CACHE:
        nc = build_nc(version)
        if not nc.is_finalized():
            nc.finalize()
        _NC_CACHE[version] = nc
    return _NC_CACHE[version]


def run(inputs, trace=False, version=None, **kwargs):
    """Run on the 8 NeuronCores; returns BassKernelResults."""
    in_maps = shard_inputs(inputs["x"], inputs["filt_w"], version=version)
    nc = _get_nc(version)
    return run_bass_kernel_spmd(
        nc, in_maps, core_ids=list(range(N_CORES)), trace=trace, **kwargs
    )


def kernel(x, filt_w):
    res = run({"x": x, "filt_w": filt_w})
    return unshard_output(res.results)


# revision 47
# speedup vs baseline: 1.0255x; 1.0255x over previous
"""Trainium2 Bass kernel for nn_DynamicConv (per-pixel dynamic 5x5 conv, 8 heads).

Reference computation (per batch image b):
    f[i, j, :]  = sum_c x[b, c, i, j] * filt_w[c, :]          # (56,56,200)
    out[c, i, j] = sum_{kh,kw} xpad[c, i+kh, j+kw] * f[i, j, kh, kw, c//24]

Sharding: data-parallel over batch, but each core takes 2 images x one
28-column half of the width so that 112 of 128 SBUF partitions carry
(row, image) pairs: partition q = 2*row + img.  Compute-engine APs must
start at partition 0 (quad-aligned), so the five kh row shifts are
materialized as five separately-laid-out DRAM loads x_d0..x_d4
(x_dk[q, c, jp] = xpad[img, c, i+k, jp]); the kw shifts are free-dim
offsets.

VERSION=8 (fp16 pipeline, measured 108us vs the 256us fp32 v4 baseline;
v4 kept for reference):
 - Everything 16-bit on chip: x loads, filt weights, generated filters,
   products, and the output store are float16; only PSUM accumulation
   (exact) is fp32.  DVE tensor ops hit the 2x_1p fast path (2-byte
   packed operands, even innermost counts) and PE matmuls run 1
   cycle/col.  Measured absmax rel err 6.5e-4 (harness gate 2e-2).
 - DMA: transfers are chained in need order (chain_iter_dep) because
   concurrent transfers round-robin at ~1/N bandwidth each; x_gen is
   split unevenly (8/20 j columns) so its small first piece exits the
   round-robin early and filter-gen starts on it.
 - PE filter-gen: per output column j, the (96ch x 128px) channel-major
   x slice is stationary (128 cols => automatic Fast Weight Load)
   against fw columns, fp32 PSUM batched 4 j per tile so evacuation
   writes contiguous fp16 runs (scattered stride-28 writes cost
   ~5ns/elem).  Split by head-half: phase A (heads 0-3) gates the conv
   start, phase B interleaves with the first conv half on the PE's
   slack (region-level dep tracking).
 - DVE conv: one (head, ch, j) product tap per channel-half (walrus
   limits TensorTensor to 3 free dims, and the f-broadcast across the
   24 channels of a head needs its own step-0 dim): 50 tensor_muls of
   2688 elems at ~1514ns, 75.7us total -- the kernel's floor.
 - PE sums the 25 taps per half into PSUM via fp16 identity matmuls
   (6 x 448-col chunks, 112-row identity padded to 128 cols for FWL);
   ACT/DVE split the evacuation; output chunks DMA out as they land.
 - GPSIMD offload rejected twice (see GJ/GT notes below).
"""

import numpy as np

import concourse.bacc as bacc
import concourse.mybir as mybir
import concourse.tile as tile
from concourse.bass_utils import run_bass_kernel_spmd

B, C, H, W = 8, 192, 56, 56
K, HEADS = 5, 8
CG = C // HEADS            # 24 channels per head
FCOLS = K * K * HEADS      # 200 filter-gen outputs per pixel
WH = 28                    # columns per core (half width)
JP = WH + 4                # padded columns held in SBUF
P_O = 2 * H                # 112 partitions carrying (row, img) pairs
JCH = 7                    # filter-gen j-columns per chunk
N_CORES = 8

F32 = mybir.dt.float32
F16 = mybir.dt.float16

VERSION = 8

F32R = mybir.dt.float32r
HHEADS = HEADS // 2        # heads per channel-half
NCH = 6                    # PSUM chunks per half
CHF = 96 * WH // NCH       # 448 fp32 per chunk = 16 channels x 28 cols
# GPSIMD product offload, tried twice and reverted both times:
#  - j-slices: ~1.6us fixed overhead per op (a 480-elem op cost 2.6us), and
#    the DVE's odd-width remainder lost its 2x_1p fast path (1520 -> 2607ns).
#  - whole taps: the op itself matched the model (6.2-6.4us for 2688 elems),
#    but concurrent GPSIMD SBUF reads slowed EVERY DVE multiply by ~28%
#    (1517 -> 1938 ns) via SBUF port contention - a large net loss.
GJ = 0                     # j columns of each product tap done on GPSIMD
GT = 0                     # whole taps per half done on GPSIMD (kh=0 taps)


def build_nc(version=None):
    if version is None:
        version = VERSION
    if version >= 8:
        return build_nc_v8()
    return build_nc_v4(version)


def build_nc_v8():
    nc = bacc.Bacc(None)

    xd_in = [
        nc.dram_tensor(f"x_d{k}", [P_O, C, JP], F16, kind="ExternalInput")
        for k in range(K)
    ]
    # Stationary operands carry 128 columns (output partitions) so the
    # compiler's Fast Weight Load kicks in (requires NumWeights==128 and a
    # non-fp32 dtype); the extra columns are zero and the extra PSUM
    # partitions are never read.  j-major layout so the two chained
    # j-half DMAs are one contiguous run per partition.
    xg_in = nc.dram_tensor("x_gen", [96, WH, 2, 128], F16, kind="ExternalInput")
    fw_in = nc.dram_tensor("fw_pk", [96, 2, FCOLS], F16, kind="ExternalInput")
    id_in = nc.dram_tensor("identh", [P_O, 128], F16, kind="ExternalInput")
    out_d = nc.dram_tensor("out_sbl", [P_O, C, WH], F16, kind="ExternalOutput")

    with tile.TileContext(nc) as tc:
        with (
            tc.tile_pool(name="big", bufs=1) as big,
            tc.tile_pool(name="pr", bufs=4) as pr,
            tc.tile_pool(name="ps_f", bufs=2, space="PSUM") as ps_f,
            tc.tile_pool(name="ps_a", bufs=NCH, space="PSUM") as ps_a,
        ):
            fw_sb = big.tile([96, 2, FCOLS], F16)
            xg = big.tile([96, WH, 2, 128], F16)
            xd = [
                big.tile([P_O, C, JP], F16, tag=f"xd{k}", name=f"xd{k}")
                for k in range(K)
            ]
            f_sb = big.tile([P_O, K * K, HEADS, WH], F16)
            acc = big.tile([P_O, C, WH], F16)
            ident = big.tile([P_O, 128], F16)

            # SDMA engines round-robin between logical queues at packet
            # granularity, so concurrent transfers each get ~1/N bandwidth
            # and equal-size splits all finish together, while chaining
            # costs ~2.7us dead time per hop (completion sem + reissue).
            # So: split x_gen UNEVENLY (8/20 columns), both issued at t=0 -
            # round-robin lets the small first piece exit early (filter-gen
            # phase A starts on it) without delaying the total - and chain
            # the five xd loads in need order behind the small piece.
            XJ0 = 8            # covers the first two 4-column filt-gen groups
                               # (12 and chaining xd0 behind all of xg both
                               # measured ~2-4us worse: this DMA schedule is
                               # a tight local minimum)
            nc.sync.dma_start(fw_sb[:], fw_in[:])
            nc.sync.dma_start(ident[:], id_in[:])
            i_xg0 = nc.sync.dma_start(xg[:, :XJ0], xg_in[:, :XJ0])
            tc.chain_iter_dep("dma_b1", i_xg0.ins)
            tc.chain_iter_dep("dma_b2", i_xg0.ins)
            nc.sync.dma_start(xg[:, XJ0:], xg_in[:, XJ0:])
            # xd0 alone first (first conv tap gates on it; chaining it
            # behind ALL of xg measured worse), then two parallel branches
            # {xd1->xd3} and {xd2->xd4}: fewer 2.7us chain hops while each
            # arrival still beats its first tap.
            i_xd0 = nc.sync.dma_start(xd[0][:], xd_in[0][:])
            tc.chain_iter_dep("dma_b1", i_xd0.ins)
            tc.chain_iter_dep("dma_b2", i_xd0.ins)
            for k, key in [(1, "dma_b1"), (2, "dma_b2"), (3, "dma_b1"), (4, "dma_b2")]:
                i_k = nc.sync.dma_start(xd[k][:], xd_in[k][:])
                tc.chain_iter_dep(key, i_k.ins)

            # PE warm-up: dummy matmuls on the already-resident identity
            # during the x_gen DMA wait pre-ramp the HAM clock gate (full
            # 2.4GHz needs ~3us of continuous PE busy), so filter-gen
            # phase A doesn't run at the 1.2GHz mid p-state.  Sized to end
            # just before x_gen's first piece + completion sem (~12.4us):
            # 22 MMs measured ending at 13.0us, gating phase A behind the
            # FIFO PE queue, so 18.
            for w in range(18):
                wps = ps_f.tile([128, 128], F32, tag="fps", name=f"warm{w}")
                nc.tensor.matmul(wps[:], ident[:], ident[:], start=True, stop=True)

            # ---- filter generation: f[q, kl, h, j] = sum_c x[c, q, j] * fw[c, kl*8+h]
            # FJB j columns accumulate into one PSUM tile (j-slot major, so
            # each matmul writes a contiguous run) and evacuate together:
            # the f_sb write then lands as contiguous fp16 runs instead of
            # lone stride-28 elements, which dominated v9's head (~1us per
            # single-column copy).
            #
            # Filter generation is split by head-half: phase A (heads 0-3)
            # gates the conv start; phase B (heads 4-7) interleaves with the
            # first conv half on the PE's slack (region-level dependency
            # tracking lets hh=0 products start on phase A alone).
            FJB = 4            # j columns batched per filter-gen PSUM tile
            def filt_gen(hp, jc, evac_eng):
                h0 = hp * HHEADS
                fps = ps_f.tile([128, FJB, K * K * HHEADS], F32, tag="fps")
                fwv = fw_sb[:].rearrange("c k (kl h) -> c k kl h", kl=K * K)
                for js in range(FJB):
                    for ck in range(2):
                        nc.tensor.matmul(
                            fps[:, js, :],
                            xg[:, FJB * jc + js, ck, :],  # (96 ch, 128 px) stationary
                            fwv[:, ck, :, h0 : h0 + HHEADS],  # (96 ch, 25*4)
                            start=(ck == 0),
                            stop=(ck == 1),
                        )
                src = fps[:P_O].rearrange("p s (kl h) -> p kl h s", kl=K * K)
                dst = f_sb[:, :, h0 : h0 + HHEADS, FJB * jc : FJB * (jc + 1)]
                if evac_eng == "act":
                    nc.scalar.copy(dst, src)
                else:
                    nc.vector.tensor_copy(dst, src)

            # the LAST group's evac goes on the DVE: the first conv product
            # (also DVE) then queues right behind it with no cross-engine
            # semaphore hop
            for jc in range(WH // FJB):
                filt_gen(0, jc, "act" if jc % 2 == 1 else "dve")

            # ---- conv: DVE computes one (h, g, j) product tap per half
            # (ISA limit: TensorTensor free APs are at most 3-D, and the
            # g-broadcast of f needs its own step-0 dim); PE accumulates
            # all 25 taps into PSUM.  GT whole taps per half run on the
            # otherwise-idle GPSIMD into dedicated tiles; their PE
            # accumulation is spliced late in each half's stream so a
            # slower-than-expected GPSIMD never stalls the PE.
            def tap_views(hh, kl):
                kh, kw = divmod(kl, K)
                c0 = hh * 96
                xin = xd[kh][:, c0 : c0 + 96, kw : kw + WH]
                xin4 = xin.rearrange("p (h g) j -> p h g j", h=HHEADS)
                fv = (
                    f_sb[:, kl, hh * HHEADS : (hh + 1) * HHEADS, :]
                    .unsqueeze(2)
                    .broadcast_to([P_O, HHEADS, CG, WH])
                )
                return xin4, fv

            gp_taps = list(range(GT))        # kh=0 taps: ready earliest
            gp_prod = {}

            def gp_issue(hh):
                for kl in gp_taps:
                    t = big.tile(
                        [P_O, HHEADS, CG, WH], F16, name=f"gprod{hh}_{kl}"
                    )
                    xin4, fv = tap_views(hh, kl)
                    nc.gpsimd.tensor_mul(t[:], xin4, fv)
                    gp_prod[(hh, kl)] = t

            def accum(accps, prod, first, last):
                pm = prod[:].rearrange("p h g j -> p (h g j)")
                for b in range(NCH):
                    nc.tensor.matmul(
                        accps[b][:],
                        ident[:],
                        pm[:, b * CHF : (b + 1) * CHF],
                        start=first,
                        stop=last,
                    )

            gp_issue(0)
            for hh in range(2):
                c0 = hh * 96
                accps = [
                    ps_a.tile([128, CHF], F32, tag="accps", name=f"accps{hh}_{b}")
                    for b in range(NCH)
                ]
                dve_taps = [kl for kl in range(K * K) if kl not in gp_taps]
                splice_at = 18 if hh == 0 else 16
                for idx, kl in enumerate(dve_taps):
                    xin4, fv = tap_views(hh, kl)
                    prod = pr.tile(
                        [P_O, HHEADS, CG, WH], F16, tag="prod",
                        name=f"prod{hh}_{kl}",
                    )
                    nc.vector.tensor_mul(prod[:], xin4, fv)

                    # interleave phase-B filter generation (heads 4-7) with
                    # the first conv half, every other tap so the added PE
                    # work (8 matmuls/group) stays under the DVE tap pace;
                    # its evacs stay off the busy DVE
                    if hh == 0 and idx % 2 == 0 and idx // 2 < WH // FJB:
                        filt_gen(1, idx // 2, "act")

                    accum(accps, prod, idx == 0, idx == len(dve_taps) - 1)
                    if idx == splice_at:
                        for kl_g in gp_taps:
                            accum(accps, gp_prod[(hh, kl_g)], False, False)
                # phase B is fully emitted by the end of the hh=0 loop, so
                # the hh=1 GPSIMD products (which read phase-B f regions)
                # can only be issued here.
                if hh == 0:
                    gp_issue(1)
                # PSUM evacuation per 16-channel chunk, store per 48-channel
                # group: 16-channel chunk DMAs were 112 descriptors of 896B
                # each and the final six drained for ~4us; a 48-channel
                # group is one contiguous 2688B run per partition (6x fewer
                # descriptors).  The final half interleaves ACT/DVE per
                # group so each group's three evacs (and so its DMA) finish
                # as early as possible.
                # final half: DVE (free after its last product, and faster
                # per copy) takes b0/b2 and crucially b5 - the copy gating
                # the last output DMA
                dve_evacs = {0, 2, 5}
                for g in range(2):
                    for i in range(NCH // 2):
                        b = g * (NCH // 2) + i
                        dst = acc[:, c0 + b * 16 : c0 + (b + 1) * 16, :]
                        src = accps[b][:P_O].rearrange("p (c j) -> p c j", j=WH)
                        if hh == 1 and b in dve_evacs:
                            nc.vector.tensor_copy(dst, src)
                        else:
                            nc.scalar.copy(dst, src)
                    lo = c0 + g * 48
                    nc.sync.dma_start(
                        out_d[:, lo : lo + 48, :], acc[:, lo : lo + 48, :]
                    )

    return nc


def build_nc_v4(version=4):
    nc = bacc.Bacc(None)

    xd_in = [
        nc.dram_tensor(f"x_d{k}", [P_O, C, JP], F32, kind="ExternalInput")
        for k in range(K)
    ]
    xg_in = nc.dram_tensor("x_gen", [96, 2, WH, P_O], F32, kind="ExternalInput")
    fw_in = nc.dram_tensor("fw_pk", [96, 2, FCOLS], F32, kind="ExternalInput")
    id_in = nc.dram_tensor("ident", [P_O, P_O], F32R, kind="ExternalInput")
    out_d = nc.dram_tensor("out_sbl", [P_O, C, WH], F32, kind="ExternalOutput")

    with tile.TileContext(nc) as tc:
        with (
            tc.tile_pool(name="big", bufs=1) as big,
            tc.tile_pool(name="sh", bufs=2) as sh,
            tc.tile_pool(name="ps_f", bufs=2, space="PSUM") as ps_f,
            tc.tile_pool(name="ps_a", bufs=NCH, space="PSUM") as ps_a,
        ):
            xd = [
                big.tile([P_O, C, JP], F32, tag=f"xd{k}", name=f"xd{k}")
                for k in range(K)
            ]
            fw_sb = big.tile([96, 2, FCOLS], F32)
            f_sb = big.tile([P_O, K * K, HEADS, WH], F32)
            acc = big.tile([P_O, C, WH], F32)
            ident = big.tile([P_O, P_O], F32R)

            nc.sync.dma_start(ident[:], id_in[:])
            for k in range(K):
                nc.sync.dma_start(xd[k][:], xd_in[k][:])
            nc.sync.dma_start(fw_sb[:], fw_in[:])

            for jc in range(WH // JCH):
                xg = sh.tile([96, 2, JCH, P_O], F32, tag="xgprod")
                nc.sync.dma_start(xg[:], xg_in[:, :, jc * JCH : (jc + 1) * JCH, :])
                for jl in range(JCH):
                    j = jc * JCH + jl
                    fps = ps_f.tile([P_O, K * K, HEADS], F32, tag="fps")
                    for ck in range(2):
                        nc.tensor.matmul(
                            fps[:],
                            xg[:, ck, jl, :],
                            fw_sb[:, ck, :],
                            start=(ck == 0),
                            stop=(ck == 1),
                        )
                    nc.scalar.copy(f_sb[:, :, :, j], fps[:])

            for hh in range(2):
                c0 = hh * 96
                accps = [
                    ps_a.tile([P_O, CHF], F32, tag="accps", name=f"accps{hh}_{b}")
                    for b in range(NCH)
                ]
                for kl in range(K * K):
                    kh, kw = divmod(kl, K)
                    xin = xd[kh][:, c0 : c0 + 96, kw : kw + WH]
                    xin4 = xin.rearrange("p (h g) j -> p h g j", h=HHEADS)
                    fbc = (
                        f_sb[:, kl, hh * HHEADS : (hh + 1) * HHEADS, :]
                        .unsqueeze(2)
                        .broadcast_to([P_O, HHEADS, CG, WH])
                    )
                    prod = sh.tile(
                        [P_O, 96, WH], F32R, tag="xgprod", name=f"prod{hh}_{kl}",
                    )
                    p4 = prod[:].rearrange("p (h g) j -> p h g j", h=HHEADS)
                    nc.vector.tensor_mul(p4, xin4, fbc)
                    pflat = prod[:].rearrange("p c j -> p (c j)")
                    for b in range(NCH):
                        nc.tensor.matmul(
                            accps[b][:],
                            ident[:],
                            pflat[:, b * CHF : (b + 1) * CHF],
                            start=(kl == 0),
                            stop=(kl == K * K - 1),
                        )
                for b in range(NCH):
                    nc.scalar.copy(
                        acc[:, c0 + b * 16 : c0 + (b + 1) * 16, :],
                        accps[b][:].rearrange("p (c j) -> p c j", j=WH),
                    )

            nc.sync.dma_start(out_d[:], acc[:])

    return nc


def shard_inputs(x, filt_w, version=None):
    """Split full inputs into the 8 per-core input maps."""
    if version is None:
        version = VERSION
    dt = np.float16 if version >= 8 else np.float32
    x = np.ascontiguousarray(np.asarray(x, dtype=np.float32))
    fw = np.ascontiguousarray(np.asarray(filt_w, dtype=np.float32))
    fw_pk = np.ascontiguousarray(
        fw.reshape(2, 96, FCOLS).transpose(1, 0, 2)
    ).astype(dt)

    in_maps = []
    for core in range(N_CORES):
        pair, jh = divmod(core, 2)
        xs = x[2 * pair : 2 * pair + 2]           # (2, C, 56, 56)
        xpad = np.zeros((2, C, H + 4, JP), np.float32)
        lo = jh * WH - 2                           # global col of jp=0
        s_lo, s_hi = max(lo, 0), min(lo + JP, W)
        xpad[:, :, 2 : 2 + H, s_lo - lo : s_lo - lo + (s_hi - s_lo)] = xs[
            :, :, :, s_lo:s_hi
        ]
        m = {"fw_pk": fw_pk}
        if version >= 8:
            m["identh"] = np.eye(P_O, 128, dtype=np.float16)
        else:
            m["ident"] = np.eye(P_O, dtype=np.float32)
        for k in range(K):
            # x_dk[2*i+img, c, jp] = xpad[img, c, i+k, jp]
            m[f"x_d{k}"] = np.ascontiguousarray(
                xpad[:, :, k : k + H, :].transpose(2, 0, 1, 3).reshape(P_O, C, JP)
            ).astype(dt)
        # channel-major copy for filter-gen: x_gen[c96, j, ck, 2*i+img]
        xs_half = xs[:, :, :, jh * WH : (jh + 1) * WH]  # (2, C, 56, 28)
        xg = xs_half.transpose(1, 3, 2, 0).reshape(C, WH, P_O)
        if version >= 8:
            # j-major for the split DMA; pad px to 128 for FWL
            xg = xg.reshape(2, 96, WH, P_O).transpose(1, 2, 0, 3)
            xgp = np.zeros((96, WH, 2, 128), np.float32)
            xgp[:, :, :, :P_O] = xg
            xg = xgp
        else:
            xg = xg.reshape(2, 96, WH, P_O).transpose(1, 0, 2, 3)
        m["x_gen"] = np.ascontiguousarray(xg).astype(dt)
        in_maps.append(m)
    return in_maps


def unshard_output(results):
    """Reassemble the 8 per-core outputs into the full (B, C, H, W) tensor."""
    out = np.empty((B, C, H, W), np.float32)
    for core in range(N_CORES):
        pair, jh = divmod(core, 2)
        arr = np.asarray(results[core]["out_sbl"]).astype(np.float32)
        arr = arr.reshape(H, 2, C, WH)
        # arr[i, img, c, j] = out[2*pair+img, c, i, jh*28+j]
        out[2 * pair : 2 * pair + 2, :, :, jh * WH : (jh + 1) * WH] = arr.transpose(
            1, 2, 0, 3
        )
    return out


_NC_CACHE = {}


def _get_nc(version=None):
    if version is None:
        version = VERSION
    if version not in _NC_CACHE:
        nc = build_nc(version)
        if not nc.is_finalized():
            nc.finalize()
        _NC_CACHE[version] = nc
    return _NC_CACHE[version]


def run(inputs, trace=False, version=None, **kwargs):
    """Run on the 8 NeuronCores; returns BassKernelResults."""
    in_maps = shard_inputs(inputs["x"], inputs["filt_w"], version=version)
    nc = _get_nc(version)
    return run_bass_kernel_spmd(
        nc, in_maps, core_ids=list(range(N_CORES)), trace=trace, **kwargs
    )


def kernel(x, filt_w):
    res = run({"x": x, "filt_w": filt_w})
    return unshard_output(res.results)
